# revision 21
# baseline (speedup 1.0000x reference)
"""Trainium2 Bass kernel for nn_MultiHeadCrossAttention (B=4, N=2048, C=256, H=4, d=64).

Sharding: 8 cores, core c -> (batch b = c//2, query-half qh = c%2).
Each core computes full 4-head cross-attention for its 1024-query slice of
its batch, plus the residuals and output projection. No collectives; the
host slices/transposes/casts inputs per core and concatenates the outputs.

With gamma == 0 (as produced by setup_inputs), the LAM channel-attention
block is exactly the identity, so:
    out = (t2_grad + q + attn_out) @ Wproj + bproj

v3 (bf16 + dual-engine exp): all matmul operands are bf16 (PSUM
accumulation stays fp32) — fp32/fp32r matmuls paid ~2x on TRN2 for the
projection/S streams, and bf16 halved Tensor-engine busy 101us -> 81us.
The softmax exp (8.4M elements/core, only ACT and DVE can read PSUM) is
split across both engines: ACT computes exact exp for the j=0 query
blocks (plus every third chunk's j=1), DVE computes a Schraudolph-style
bf16 exp (int16 round(S*A+B), bits reinterpreted as bf16) for the rest;
the ~5% sawtooth cancels between softmax numerator and denominator,
which use the same approximation per query.  Layout:
 - t1T/t2T: (C, keys/queries) bf16.  kT/qT = W^T @ tT via PE, heads
   pair-packed (tile m holds heads 2m, 2m+1 on partition halves);
   emission interleaves kT/v/qT per DMA chunk to absorb load latency.
 - v tiles per key chunk: (128, 4*65) bf16; head h cols = [1 | v_h] so
   the softmax denominator rides the xo matmul as output row 0.
 - S^T tiles (keys on partitions, queries free) pack both heads side by
   side in the free dim; the two K=64 matmuls hit PE row groups 0/64
   (throughput is PSUM-write-port bound at 128 values/cycle).
 - unnormalized xo^T accumulates in PSUM; normalization multiplies by a
   GpSimd-broadcast reciprocal into per-head [65, Q] tiles whose row 0
   (den*recip ~= 1) is killed by a zero row in the wp_h weights — except
   wp_h[0] row 0, which carries bproj so the bias rides the projection.
 - final projection: out = x^T.T @ Wproj with K-groups [t2T+qT (2x128),
   4 per-head xon (65)], split into pass A (pair 0, overlaps pair-1
   normalization) and pass B.
All engine ops keep in/out partition bases equal (DVE/ACT lanes are
partition-locked); cross-partition moves go through GpSimd broadcast.
"""

from contextlib import ExitStack

import numpy as np

import concourse.bass as bass
import concourse.mybir as mybir
import concourse.tile as tile
from concourse import bacc
from concourse.bass_utils import run_bass_kernel_spmd

B, N, C, H, D = 4, 2048, 256, 4, 64
NCORES = 8
Q = 1024  # queries per core
SCALE = float(D) ** -0.5
FP32 = mybir.dt.float32
BF16 = mybir.dt.bfloat16
I16 = mybir.dt.int16
AF = mybir.ActivationFunctionType
ALU = mybir.AluOpType

# Schraudolph bf16 exp: bitcast(int16(round(x * EXPA + EXPB))) ~= exp(x/8)
# (the softmax 1/sqrt(d) scale is folded into EXPA; C=16 centers the
# sawtooth error for round-to-nearest, measured on hardware)
EXPA = float(128.0 / np.log(2.0)) * SCALE
EXPB = 127.0 * 128.0 - 16.0

_CACHE = {}


def build_nc():
    nc = bacc.Bacc("TRN2", target_bir_lowering=False, debug=False,
                   num_devices=NCORES)
    MDT = BF16

    t1T_d = nc.dram_tensor("t1T", [C, N], MDT, kind="ExternalInput")
    t2T_d = nc.dram_tensor("t2T", [C, Q], MDT, kind="ExternalInput")
    wq_d = nc.dram_tensor("wq", [C, C], MDT, kind="ExternalInput")
    wk_d = nc.dram_tensor("wk", [C, C], MDT, kind="ExternalInput")
    wv_d = nc.dram_tensor("wv", [C, C], MDT, kind="ExternalInput")
    wp_d = nc.dram_tensor("wp", [C, C], MDT, kind="ExternalInput")
    bpT_d = nc.dram_tensor("bpT", [C, 1], FP32, kind="ExternalInput")
    # output is stored TRANSPOSED (channels x queries); the host undoes it
    out_d = nc.dram_tensor("out", [C, Q], FP32, kind="ExternalOutput")

    with tile.TileContext(nc) as tc, ExitStack() as ctx:
        const = ctx.enter_context(tc.tile_pool(name="const", bufs=1))
        acts = ctx.enter_context(tc.tile_pool(name="acts", bufs=1))

        # ---- PE warm-up: the HAM clock gate starts at 1.2 GHz and only
        # reaches 2.4 GHz after ~3.4us of sustained PE activity.  The
        # first real matmul cannot start before ~11.5us (framework
        # preamble + input DMA latency), so burn that window on dummy
        # matmuls over a zeroed tile to enter the kT phase at full clock.
        warm = const.tile([128, 512], MDT, name="warm", tag="warm")
        nc.vector.memset(warm[:], 0.0)
        with tc.tile_pool(name="warmps", bufs=1, space="PSUM") as wpool:
            wps = wpool.tile([128, 512], FP32, name="wps", tag="wps")
            for _ in range(7):
                nc.tensor.matmul(wps[:], lhsT=warm[:, 0:128], rhs=warm[:],
                                 start=True, stop=True)

        # ---- load inputs (critical path first: wk, then t1T chunks) ----
        # DMAs are spread across BOTH HWDGE queues (sync=SP, scalar=ACT);
        # a single queue serializes at ~600ns per 128x512 chunk and left
        # the PE idle until 12.7us.  cc=0 chunks ride SP, cc=1 rides ACT.
        dmae = [nc.sync, nc.scalar]
        w_sb = {}
        for name in ("wk", "wq", "wv", "wp"):
            w_sb[name] = [const.tile([128, C], MDT, name=f"{name}{cc}",
                                     tag=f"{name}{cc}") for cc in range(2)]

        def load_w(name, dram):
            for cc in range(2):
                dmae[cc].dma_start(out=w_sb[name][cc][:],
                                   in_=dram[cc * 128:(cc + 1) * 128, :])

        t1T = [acts.tile([128, N], MDT, name=f"t1T{cc}", tag=f"t1T{cc}")
               for cc in range(2)]
        t2T = [acts.tile([128, Q], MDT, name=f"t2T{cc}", tag=f"t2T{cc}")
               for cc in range(2)]
        def load_t1(nn):
            for cc in range(2):
                dmae[cc].dma_start(
                    out=t1T[cc][:, nn * 512:(nn + 1) * 512],
                    in_=t1T_d[cc * 128:(cc + 1) * 128, nn * 512:(nn + 1) * 512])

        def load_t2(nn):
            for cc in range(2):
                dmae[cc].dma_start(
                    out=t2T[cc][:, nn * 512:(nn + 1) * 512],
                    in_=t2T_d[cc * 128:(cc + 1) * 128, nn * 512:(nn + 1) * 512])

        # queue order = consumption order: the first kT matmul needs only
        # wk + t1(0); wq/wv/wp are needed progressively later.
        load_w("wk", wk_d)
        load_t1(0)
        load_t1(1)
        load_t1(2)
        load_t1(3)
        load_t2(0)
        load_w("wq", wq_d)
        load_t2(1)
        load_w("wv", wv_d)
        load_w("wp", wp_d)

        # wp_h[h] row 0 multiplies xon[h] row 0 = den*recip ~= 1 in the
        # final projection and is kept zero to kill that row; the bias is
        # added per-partition during the transposed-output evacuation.
        wp_h = []
        for h in range(4):
            t = const.tile([65, C], MDT, name=f"wph{h}", tag=f"wph{h}")
            nc.gpsimd.memset(t[:], 0.0)
            dmae[h % 2].dma_start(out=t[1:65, :],
                                  in_=wp_d[h * 64:(h + 1) * 64, :])
            wp_h.append(t)
        bpT = []
        for ccq in range(2):
            t = const.tile([128, 1], FP32, name=f"bpT{ccq}", tag=f"bpT{ccq}")
            dmae[ccq].dma_start(out=t[:],
                                in_=bpT_d[ccq * 128:(ccq + 1) * 128, :])
            bpT.append(t)

        # ---- phase 1: projections kT, qT, v ----
        kT = [acts.tile([128, N], MDT, name=f"kT{m}", tag=f"kT{m}")
              for m in range(2)]
        qT = [acts.tile([128, Q], MDT, name=f"qT{m}", tag=f"qT{m}")
              for m in range(2)]
        v_sb = []
        for kc in range(16):
            t = acts.tile([128, 4 * 65], MDT, name=f"v{kc}", tag=f"v{kc}")
            # pre-fill with 1.0: cols 64/129/194/259 stay as the softmax
            # denominator "ones" columns; the rest is overwritten with v
            nc.gpsimd.memset(t[:], 1.0)
            v_sb.append(t)

        # attention pools (opened before projections so the first S/exp
        # pairs can be hoisted into the projection phase)
        xT = [acts.tile([128, Q], MDT, name=f"xT{m}", tag=f"xT{m}")
              for m in range(2)]
        # normalized attention outputs, one [65, Q] tile per head; row 0
        # holds den*recip ~= 1 and is killed by the zero row in wp_h
        xon = [acts.tile([65, Q], MDT, name=f"xon{h}", tag=f"xon{h}")
               for h in range(4)]
        attn_ctx = ExitStack()
        spool = attn_ctx.enter_context(
            tc.tile_pool(name="spsum", bufs=1, space="PSUM"))
        ppool2 = ctx.enter_context(tc.tile_pool(name="pexp", bufs=8))
        npool = ctx.enter_context(tc.tile_pool(name="norm", bufs=2))
        hoisted = []

        def emit_s_exp(m, kc, dve_busy=False):
            s_ts = []
            for j in range(Q // 512):
                s_t = spool.tile([128, Q], FP32, name=f"sq{j}", tag=f"sq{j}")
                for hh in range(2):
                    base = hh * 64
                    nc.tensor.matmul(
                        s_t[:, hh * 512:(hh + 1) * 512],
                        lhsT=kT[m][base:base + 64, kc * 128:(kc + 1) * 128],
                        rhs=qT[m][base:base + 64, j * 512:(j + 1) * 512],
                        start=True, stop=True)
                s_ts.append(s_t)
            # exp split across engines (GpSimd cannot read PSUM, so the
            # only softmax-capable engines are ACT and DVE): queries of
            # j=0 get the exact ACT exp (bf16 out, to match the bf16 v
            # weights in the xo matmul); queries of j=1 get
            # a Schraudolph-style bf16 exp on DVE (int16 = round(S*A+B),
            # bits reinterpreted as bf16; the ~5% sawtooth cancels
            # between softmax numerator and denominator, which use the
            # same approximation per query).
            # ACT always takes the first tile (exact exp, 1.07us); the
            # second goes to DVE's Schraudolph (1.21us) on steady-state
            # chunks where DVE has no other duty, and to ACT on chunks
            # that overlap DVE's cast / normalization work (the hoisted
            # projection-phase and pair-boundary chunks).  Per steady
            # chunk each engine then absorbs one tile, under the PE's
            # ~1.3us, so the S-tile PSUM recycle never stalls.
            pe0 = ppool2.tile([128, Q], MDT, name="pexp0", tag="pexp0")
            nc.scalar.activation(pe0[:], s_ts[0][:], AF.Exp, scale=SCALE)
            if dve_busy:
                pe1 = ppool2.tile([128, Q], MDT, name="pexp1", tag="pexp1")
                nc.scalar.activation(pe1[:], s_ts[1][:], AF.Exp, scale=SCALE)
            else:
                pe1 = ppool2.tile([128, Q], I16, name="pexp1", tag="pexp1")
                nc.vector.tensor_scalar(pe1[:], s_ts[1][:], EXPA, EXPB,
                                        op0=ALU.mult, op1=ALU.add)
            return [pe0, pe1]

        # PSUM evacuation casts alternate DVE / ACT: both engines are
        # otherwise idle in this phase and each copy is ~0.5us.
        def evac(i, out, in_):
            if i % 3 != 2:
                nc.vector.tensor_copy(out, in_)
            else:
                nc.scalar.copy(out, in_)

        with tc.tile_pool(name="ppsum", bufs=2, space="PSUM") as ppool:
            for m in range(2):
                for nn in range(N // 512):
                    ps = ppool.tile([128, 512], FP32, name="p", tag="p")
                    for cc in range(2):
                        nc.tensor.matmul(
                            ps[:],
                            lhsT=w_sb["wk"][cc][:, m * 128:(m + 1) * 128],
                            rhs=t1T[cc][:, nn * 512:(nn + 1) * 512],
                            start=(cc == 0), stop=(cc == 1))
                    evac(m * 4 + nn, kT[m][:, nn * 512:(nn + 1) * 512],
                         ps[:])
            for m in range(2):
                for nn in range(Q // 512):
                    ps = ppool.tile([128, 512], FP32, name="p", tag="p")
                    for cc in range(2):
                        nc.tensor.matmul(
                            ps[:],
                            lhsT=w_sb["wq"][cc][:, m * 128:(m + 1) * 128],
                            rhs=t2T[cc][:, nn * 512:(nn + 1) * 512],
                            start=(cc == 0), stop=(cc == 1))
                    evac(m * 2 + nn, qT[m][:, nn * 512:(nn + 1) * 512],
                         ps[:])
            hoisted.append(emit_s_exp(0, 0, dve_busy=True))
            hoisted.append(emit_s_exp(0, 1, dve_busy=True))
            hoisted.append(emit_s_exp(0, 2, dve_busy=True))
            hoisted.append(emit_s_exp(0, 3, dve_busy=True))
            for kc in range(16):
                ps = ppool.tile([128, C], FP32, name="p", tag="p")
                for cc in range(2):
                    nc.tensor.matmul(
                        ps[:],
                        lhsT=t1T[cc][:, kc * 128:(kc + 1) * 128],
                        rhs=w_sb["wv"][cc][:],
                        start=(cc == 0), stop=(cc == 1))
                v3 = v_sb[kc][:].rearrange("p (h e) -> p h e", e=65)
                evac(kc, v3[:, :, 1:65],
                     ps[:].rearrange("p (h e) -> p h e", e=64))

        def emit_xo(m, kc, xo_ps, pes):
            for j in range(Q // 512):
                for hh in range(2):
                    h = 2 * m + hh
                    rhs = pes[j][:, hh * 512:(hh + 1) * 512]
                    if rhs.dtype == I16:
                        rhs = rhs.bitcast(MDT)
                    nc.tensor.matmul(
                        xo_ps[hh][0:65, j * 512:(j + 1) * 512],
                        lhsT=v_sb[kc][:, h * 65:(h + 1) * 65],
                        rhs=rhs,
                        start=(kc == 0), stop=(kc == 15))

        xopool = attn_ctx.enter_context(
            tc.tile_pool(name="xopsum", bufs=1, space="PSUM"))

        osb = ctx.enter_context(tc.tile_pool(name="osb", bufs=2))
        o_ps = None

        for m in range(2):  # head pair (2m, 2m+1)
            nc.gpsimd.tensor_add(xT[m][:], t2T[m][:], qT[m][:])

            xo_ps = [xopool.tile([65, Q], FP32, name=f"xo{hh}", tag=f"xo{hh}")
                     for hh in range(2)]

            pending = list(hoisted)
            hoisted = []
            for kc in range(16):
                if kc >= len(pending):
                    pending.append(emit_s_exp(m, kc))
                if kc + 1 < 16 and kc + 1 >= len(pending):
                    # S/exp for the next chunk goes out before this chunk's
                    # xo so the PE refills ACT's pipeline first
                    pending.append(emit_s_exp(m, kc + 1))
                emit_xo(m, kc, xo_ps, pending[kc])
            del pending

            if m == 0:
                # keep ACT fed across the pair boundary: next pair's first
                # S/exp pairs go out before this pair's normalization chain
                hoisted.append(emit_s_exp(1, 0, dve_busy=True))
                hoisted.append(emit_s_exp(1, 1, dve_busy=True))
                hoisted.append(emit_s_exp(1, 2, dve_busy=True))
            else:
                # final-projection pass A, TRANSPOSED: accumulate
                # oT[c, q] = Wp^T (t2+q)^T + pair-0 head contributions in
                # PSUM (tags sq0/sq1, free once the kc=15 exps have read
                # them).  Channels live on partitions, so every matmul
                # streams a fat N=512 and the bias can ride evacuation as
                # a per-partition scalar.  Runs on the PE while
                # DVE/GpSimd normalize pair 1 below; pass B accumulates
                # heads 2-3 into the same PSUM after normalization.
                o_ps = [spool.tile([128, Q], FP32, name=f"oT{half}",
                                   tag=f"sq{half}") for half in range(2)]
                for half in range(2):
                    hsl = slice(half * 128, (half + 1) * 128)
                    for j in range(Q // 512):
                        out_sl = o_ps[half][:, j * 512:(j + 1) * 512]
                        for cc in range(2):
                            nc.tensor.matmul(
                                out_sl,
                                lhsT=w_sb["wp"][cc][:, hsl],
                                rhs=xT[cc][:, j * 512:(j + 1) * 512],
                                start=(cc == 0), stop=False)
                        for h in range(2):
                            nc.tensor.matmul(
                                out_sl,
                                lhsT=wp_h[h][:, hsl],
                                rhs=xon[h][:, j * 512:(j + 1) * 512],
                                start=False, stop=False)

            # normalize: row 0 of xo_ps[hh] = sum_k exp(S)
            for hh in range(2):
                recip = npool.tile([1, Q], FP32, name=f"recip{hh}",
                                   tag=f"recip{hh}")
                nc.vector.reciprocal_approx_fast(recip[:, :],
                                                 xo_ps[hh][0:1, :])
                bc_sb = npool.tile([65, Q], FP32, name=f"bc{hh}",
                                   tag=f"bc{hh}")
                nc.gpsimd.partition_broadcast(bc_sb[:], recip[:])
                nc.vector.tensor_mul(xon[2 * m + hh][:], xo_ps[hh][0:65, :],
                                     bc_sb[:])

        # final-projection pass B: accumulate pair-1 heads into the
        # transposed PSUM accumulators, then evacuate with the bias added
        # per-partition (DVE and ACT alternate) and store oT.
        for half in range(2):
            hsl = slice(half * 128, (half + 1) * 128)
            for j in range(Q // 512):
                out_sl = o_ps[half][:, j * 512:(j + 1) * 512]
                for h in range(2, 4):
                    nc.tensor.matmul(
                        out_sl,
                        lhsT=wp_h[h][:, hsl],
                        rhs=xon[h][:, j * 512:(j + 1) * 512],
                        start=False, stop=(h == 3))
                o_sb = osb.tile([128, 512], FP32, name="o", tag=f"o{j}")
                if j == 0:
                    nc.vector.tensor_scalar_add(o_sb[:], out_sl, bpT[half][:])
                else:
                    nc.scalar.activation(o_sb[:], out_sl, AF.Identity,
                                         bias=bpT[half][:])
                dmae[(half + j) % 2].dma_start(
                    out=out_d[hsl, j * 512:(j + 1) * 512],
                    in_=o_sb[:])

        attn_ctx.close()

    nc.finalize()
    return nc


def _get_nc():
    if "nc" not in _CACHE:
        _CACHE["nc"] = build_nc()
    return _CACHE["nc"]


def _bf16(a):
    import ml_dtypes

    return np.ascontiguousarray(a.astype(ml_dtypes.bfloat16))


def make_in_maps(t2_grad, t1, Wq, Wkv, Wproj, bproj):
    t2 = np.asarray(t2_grad, dtype=np.float32)
    t1 = np.asarray(t1, dtype=np.float32)
    wq = _bf16(np.asarray(Wq, dtype=np.float32))
    wk = _bf16(np.ascontiguousarray(Wkv[:, :C], dtype=np.float32))
    wv = _bf16(np.ascontiguousarray(Wkv[:, C:], dtype=np.float32))
    wp = _bf16(np.asarray(Wproj, dtype=np.float32))
    bpT = np.ascontiguousarray(
        np.asarray(bproj, dtype=np.float32).reshape(C, 1))
    in_maps = []
    for c in range(NCORES):
        b, qh = c // 2, c % 2
        in_maps.append({
            "t1T": _bf16(t1[b].T),
            "t2T": _bf16(t2[b].T[:, qh * Q:(qh + 1) * Q]),
            "wq": wq, "wk": wk, "wv": wv, "wp": wp, "bpT": bpT,
        })
    return in_maps


def kernel(t2_grad, t1, Wq, Wkv, Wproj, bproj, gamma, _trace=False,
           _use_fp32r=True):
    gamma = np.asarray(gamma)
    if float(np.abs(gamma).max()) != 0.0:
        # LAM block is only the identity for gamma == 0; fall back to a
        # host reference for the general case (not exercised by the
        # reference setup_inputs, which fixes gamma = 0).
        return _host_reference(t2_grad, t1, Wq, Wkv, Wproj, bproj, gamma)

    nc = _get_nc()
    in_maps = make_in_maps(t2_grad, t1, Wq, Wkv, Wproj, bproj)
    res = run_bass_kernel_spmd(nc, in_maps, list(range(NCORES)), trace=_trace)
    out = np.empty((B, N, C), dtype=np.float32)
    for c in range(NCORES):
        b, qh = c // 2, c % 2
        out[b, qh * Q:(qh + 1) * Q, :] = res.results[c]["out"].T
    if _trace:
        _CACHE["last_result"] = res
    return out


def _host_reference(t2_grad, t1, Wq, Wkv, Wproj, bproj, gamma):
    t2 = np.asarray(t2_grad, dtype=np.float64)
    t1 = np.asarray(t1, dtype=np.float64)
    Wq = np.asarray(Wq, dtype=np.float64)
    Wkv = np.asarray(Wkv, dtype=np.float64)
    Wproj = np.asarray(Wproj, dtype=np.float64)
    bproj = np.asarray(bproj, dtype=np.float64)
    g = float(np.asarray(gamma).reshape(-1)[0])
    q = (t2 @ Wq).reshape(B, N, H, D).transpose(0, 2, 1, 3)
    kv = (t1 @ Wkv).reshape(B, N, 2, H, D).transpose(2, 0, 3, 1, 4)
    k, v = kv[0], kv[1]
    s = np.einsum('bhnd,bhmd->bhnm', q, k) * SCALE
    s = s - s.max(axis=-1, keepdims=True)
    p = np.exp(s)
    p /= p.sum(axis=-1, keepdims=True)
    x = np.einsum('bhnm,bhmd->bhnd', p, v)
    xp = x.transpose(0, 3, 1, 2).reshape(B, D, H * N)
    energy = xp @ xp.transpose(0, 2, 1)
    energy = energy - energy.max(axis=-1, keepdims=True)
    att = np.exp(energy)
    att /= att.sum(axis=-1, keepdims=True)
    lam_out = (att @ xp).reshape(B, D, H, N)
    lam_out = g * lam_out + xp.reshape(B, D, H, N)
    x = lam_out.transpose(0, 2, 3, 1)
    xo = x.transpose(0, 2, 1, 3).reshape(B, N, C) \
        + q.transpose(0, 2, 1, 3).reshape(B, N, C)
    return ((t2 + xo) @ Wproj + bproj).astype(np.float32)



# revision 29
# speedup vs baseline: 1.0454x; 1.0454x over previous
"""Trainium2 Bass kernel for nn_MultiHeadCrossAttention (B=4, N=2048, C=256, H=4, d=64).

Sharding: 8 cores, core c -> (batch b = c//2, query-half qh = c%2).
Each core computes full 4-head cross-attention for its 1024-query slice of
its batch, plus the residuals and output projection. No collectives; the
host slices/transposes/casts inputs per core and concatenates the outputs.

With gamma == 0 (as produced by setup_inputs), the LAM channel-attention
block is exactly the identity, so:
    out = (t2_grad + q + attn_out) @ Wproj + bproj

v3 (bf16 + dual-engine exp): all matmul operands are bf16 (PSUM
accumulation stays fp32) — fp32/fp32r matmuls paid ~2x on TRN2 for the
projection/S streams, and bf16 halved Tensor-engine busy 101us -> 81us.
The softmax exp (8.4M elements/core, only ACT and DVE can read PSUM) is
split across both engines: ACT computes exact exp for the j=0 query
blocks (plus every third chunk's j=1), DVE computes a Schraudolph-style
bf16 exp (int16 round(S*A+B), bits reinterpreted as bf16) for the rest;
the ~5% sawtooth cancels between softmax numerator and denominator,
which use the same approximation per query.  Layout:
 - t1T/t2T: (C, keys/queries) bf16.  kT/qT = W^T @ tT via PE, heads
   pair-packed (tile m holds heads 2m, 2m+1 on partition halves);
   emission interleaves kT/v/qT per DMA chunk to absorb load latency.
 - v tiles per key chunk: (128, 4*65) bf16; head h cols = [1 | v_h] so
   the softmax denominator rides the xo matmul as output row 0.
 - S^T tiles (keys on partitions, queries free) pack both heads side by
   side in the free dim; the two K=64 matmuls hit PE row groups 0/64
   (throughput is PSUM-write-port bound at 128 values/cycle).
 - unnormalized xo^T accumulates in PSUM; normalization multiplies by a
   GpSimd-broadcast reciprocal into per-head [65, Q] tiles whose row 0
   (den*recip ~= 1) is killed by a zero row in the wp_h weights — except
   wp_h[0] row 0, which carries bproj so the bias rides the projection.
 - final projection: out = x^T.T @ Wproj with K-groups [t2T+qT (2x128),
   4 per-head xon (65)], split into pass A (pair 0, overlaps pair-1
   normalization) and pass B.
All engine ops keep in/out partition bases equal (DVE/ACT lanes are
partition-locked); cross-partition moves go through GpSimd broadcast.
"""

from contextlib import ExitStack

import numpy as np

import concourse.bass as bass
import concourse.mybir as mybir
import concourse.tile as tile
from concourse import bacc
from concourse.bass_utils import run_bass_kernel_spmd

B, N, C, H, D = 4, 2048, 256, 4, 64
NCORES = 8
Q = 1024  # queries per core
SCALE = float(D) ** -0.5
FP32 = mybir.dt.float32
BF16 = mybir.dt.bfloat16
I16 = mybir.dt.int16
AF = mybir.ActivationFunctionType
ALU = mybir.AluOpType

# Schraudolph fp8e5m2 exp: bitcast(uint8(round(x * EXPA8 + EXPB8)))
# ~= c * exp(x/8), 4 mantissa steps per octave so the u8 bit range spans
# +-10.9 sigma of S/8 (no reachable NaN/inf/zero; +-11.6% sawtooth that
# cancels in the softmax ratio, remaining error ~ saw * sqrt(sum p^2)).
# The ACT exact-exp tiles are fp8e4m3 with exp(x/8 - EXPSHIFT) instead
# (higher precision, needs the shift to stay inside e4m3 range); the
# per-engine scale factors cancel per query between numerator and
# denominator.
EXPSHIFT = 3.0
EXPA8 = float(4.0 / np.log(2.0)) * SCALE
EXPB8 = 62.5

_CACHE = {}


def build_nc():
    nc = bacc.Bacc("TRN2", target_bir_lowering=False, debug=False,
                   num_devices=NCORES)
    MDT = BF16

    t1T_d = nc.dram_tensor("t1T", [C, N], MDT, kind="ExternalInput")
    t2T_d = nc.dram_tensor("t2T", [C, Q], MDT, kind="ExternalInput")
    wq_d = nc.dram_tensor("wq", [C, C], MDT, kind="ExternalInput")
    wk_d = nc.dram_tensor("wk", [C, C], MDT, kind="ExternalInput")
    wv_d = nc.dram_tensor("wv", [C, C], MDT, kind="ExternalInput")
    wp_d = nc.dram_tensor("wp", [C, C], MDT, kind="ExternalInput")
    bpT_d = nc.dram_tensor("bpT", [C, 1], FP32, kind="ExternalInput")
    # output is stored TRANSPOSED (channels x queries); the host undoes it
    out_d = nc.dram_tensor("out", [C, Q], FP32, kind="ExternalOutput")

    with tile.TileContext(nc) as tc, ExitStack() as ctx:
        const = ctx.enter_context(tc.tile_pool(name="const", bufs=1))
        acts = ctx.enter_context(tc.tile_pool(name="acts", bufs=1))

        # ---- PE warm-up: the HAM clock gate starts at 1.2 GHz and only
        # reaches 2.4 GHz after ~3.4us of sustained PE activity.  The
        # first real matmul cannot start before ~11.5us (framework
        # preamble + input DMA latency), so burn that window on dummy
        # matmuls over a zeroed tile to enter the kT phase at full clock.
        warm = const.tile([128, 512], MDT, name="warm", tag="warm")
        nc.vector.memset(warm[:], 0.0)
        with tc.tile_pool(name="warmps", bufs=1, space="PSUM") as wpool:
            wps = wpool.tile([128, 512], FP32, name="wps", tag="wps")
            for _ in range(7):
                nc.tensor.matmul(wps[:], lhsT=warm[:, 0:128], rhs=warm[:],
                                 start=True, stop=True)

        # ---- load inputs (critical path first: wk, then t1T chunks) ----
        # DMAs are spread across BOTH HWDGE queues (sync=SP, scalar=ACT);
        # a single queue serializes at ~600ns per 128x512 chunk and left
        # the PE idle until 12.7us.  cc=0 chunks ride SP, cc=1 rides ACT.
        dmae = [nc.sync, nc.scalar]
        w_sb = {}
        for name in ("wk", "wq", "wv", "wp"):
            w_sb[name] = [const.tile([128, C], MDT, name=f"{name}{cc}",
                                     tag=f"{name}{cc}") for cc in range(2)]

        def load_w(name, dram):
            for cc in range(2):
                dmae[cc].dma_start(out=w_sb[name][cc][:],
                                   in_=dram[cc * 128:(cc + 1) * 128, :])

        t1T = [acts.tile([128, N], MDT, name=f"t1T{cc}", tag=f"t1T{cc}")
               for cc in range(2)]
        t2T = [acts.tile([128, Q], MDT, name=f"t2T{cc}", tag=f"t2T{cc}")
               for cc in range(2)]
        def load_t1(nn):
            for cc in range(2):
                dmae[cc].dma_start(
                    out=t1T[cc][:, nn * 512:(nn + 1) * 512],
                    in_=t1T_d[cc * 128:(cc + 1) * 128, nn * 512:(nn + 1) * 512])

        def load_t2(nn):
            for cc in range(2):
                dmae[cc].dma_start(
                    out=t2T[cc][:, nn * 512:(nn + 1) * 512],
                    in_=t2T_d[cc * 128:(cc + 1) * 128, nn * 512:(nn + 1) * 512])

        # queue order = consumption order: the first kT matmul needs only
        # wk + t1(0); wq/wv/wp are needed progressively later.
        load_w("wk", wk_d)
        load_t1(0)
        load_t1(1)
        load_t1(2)
        load_t1(3)
        load_t2(0)
        load_w("wq", wq_d)
        load_t2(1)
        load_w("wv", wv_d)
        load_w("wp", wp_d)

        # wp_h[h] row 0 multiplies xon[h] row 0 = den*recip ~= 1 in the
        # final projection and is kept zero to kill that row; the bias is
        # added per-partition during the transposed-output evacuation.
        wp_h = []
        for h in range(4):
            t = const.tile([65, C], MDT, name=f"wph{h}", tag=f"wph{h}")
            nc.gpsimd.memset(t[:], 0.0)
            dmae[h % 2].dma_start(out=t[1:65, :],
                                  in_=wp_d[h * 64:(h + 1) * 64, :])
            wp_h.append(t)
        bpT = []
        for ccq in range(2):
            t = const.tile([128, 1], FP32, name=f"bpT{ccq}", tag=f"bpT{ccq}")
            dmae[ccq].dma_start(out=t[:],
                                in_=bpT_d[ccq * 128:(ccq + 1) * 128, :])
            bpT.append(t)

        # ---- phase 1: projections kT, qT, v ----
        kT = [acts.tile([128, N], MDT, name=f"kT{m}", tag=f"kT{m}")
              for m in range(2)]
        qT = [acts.tile([128, Q], MDT, name=f"qT{m}", tag=f"qT{m}")
              for m in range(2)]
        # fp8 DoubleRow v tiles, one per 256-key super-chunk: layout
        # [128p, ko(2), h(4), 80] where ko indexes the two 128-key
        # sub-chunks contracted together, col 0 of each head's 80-block
        # is the softmax-denominator "ones" column, cols 1-64 hold v and
        # 65-79 are zero pad (DoubleRow needs the ko stride 16B-aligned).
        FP8 = mybir.dt.float8e4
        FP8R = mybir.dt.float8e5
        U8 = mybir.dt.uint8
        v_sb = []
        for sc in range(8):
            t = acts.tile([128, 2 * 4 * 80], FP8, name=f"v{sc}", tag=f"v{sc}")
            nc.gpsimd.memset(t[:], 0.0)
            v4 = t[:].rearrange("p (ko h e) -> p ko h e", ko=2, h=4)
            for ko in range(2):
                nc.gpsimd.memset(v4[:, ko, :, 0:1], 1.0)
            v_sb.append(t)

        # attention pools (opened before projections so the first S/exp
        # pairs can be hoisted into the projection phase)
        xT = [acts.tile([128, Q], MDT, name=f"xT{m}", tag=f"xT{m}")
              for m in range(2)]
        # normalized attention outputs, one [65, Q] tile per head; row 0
        # holds den*recip ~= 1 and is killed by the zero row in wp_h
        xon = [acts.tile([65, Q], MDT, name=f"xon{h}", tag=f"xon{h}")
               for h in range(4)]
        attn_ctx = ExitStack()
        spool = attn_ctx.enter_context(
            tc.tile_pool(name="spsum", bufs=1, space="PSUM"))
        ppool2 = ctx.enter_context(tc.tile_pool(name="pexp", bufs=3))
        npool = ctx.enter_context(tc.tile_pool(name="norm", bufs=2))
        # exp outputs live in fp8 super tiles [128p, ko(2), q(1024)], one
        # per (j-block, super-chunk); the two ko slices are written by
        # the exps of consecutive key chunks and contracted together by
        # the DoubleRow xo matmul.
        sup_map = {}
        emitted = {0: 0, 1: 0}
        expshift_sb = const.tile([128, 1], FP32, name="expshift",
                                 tag="expshift")
        nc.vector.memset(expshift_sb[:], -EXPSHIFT)

        def emit_s_exp(m, kc, dve_busy=False):
            s_ts = []
            for j in range(Q // 512):
                s_t = spool.tile([128, Q], FP32, name=f"sq{j}", tag=f"sq{j}")
                for hh in range(2):
                    base = hh * 64
                    nc.tensor.matmul(
                        s_t[:, hh * 512:(hh + 1) * 512],
                        lhsT=kT[m][base:base + 64, kc * 128:(kc + 1) * 128],
                        rhs=qT[m][base:base + 64, j * 512:(j + 1) * 512],
                        start=True, stop=True)
                s_ts.append(s_t)
            # exp split across engines (GpSimd cannot read PSUM, so the
            # only softmax-capable engines are ACT and DVE): j=0 queries
            # get the exact ACT exp (fp8 out, shifted by -EXPSHIFT to fit
            # fp8 range; the shift cancels in the softmax ratio), j=1
            # queries a Schraudolph fp8 exp on DVE — except on chunks
            # that overlap DVE's cast / normalization work (hoisted
            # projection-phase and pair-boundary chunks), where ACT takes
            # both tiles.
            sc, ko = kc // 2, kc % 2
            if (m, sc) not in sup_map:
                sup_map[(m, sc)] = [
                    ppool2.tile([128, 2 * Q], FP8 if j == 0 else FP8R,
                                name=f"psup{j}", tag=f"psup{j}")
                    for j in range(2)]
            sup = sup_map[(m, sc)]
            s3 = [t[:].rearrange("p (ko q) -> p ko q", ko=2) for t in sup]
            nc.scalar.activation(s3[0][:, ko, :], s_ts[0][:], AF.Exp,
                                 scale=SCALE, bias=expshift_sb[:])
            if dve_busy:
                nc.scalar.activation(s3[1][:, ko, :], s_ts[1][:], AF.Exp,
                                     scale=SCALE, bias=expshift_sb[:])
            else:
                nc.vector.tensor_scalar(s3[1][:, ko, :].bitcast(U8),
                                        s_ts[1][:], EXPA8, EXPB8,
                                        op0=ALU.mult, op1=ALU.add)
            emitted[m] += 1

        # PSUM evacuation casts alternate DVE / ACT: both engines are
        # otherwise idle in this phase and each copy is ~0.5us.
        def evac(i, out, in_):
            if i % 3 != 2:
                nc.vector.tensor_copy(out, in_)
            else:
                nc.scalar.copy(out, in_)

        with tc.tile_pool(name="ppsum", bufs=2, space="PSUM") as ppool:
            for m in range(2):
                for nn in range(N // 512):
                    ps = ppool.tile([128, 512], FP32, name="p", tag="p")
                    for cc in range(2):
                        nc.tensor.matmul(
                            ps[:],
                            lhsT=w_sb["wk"][cc][:, m * 128:(m + 1) * 128],
                            rhs=t1T[cc][:, nn * 512:(nn + 1) * 512],
                            start=(cc == 0), stop=(cc == 1))
                    evac(m * 4 + nn, kT[m][:, nn * 512:(nn + 1) * 512],
                         ps[:])
            for m in range(2):
                for nn in range(Q // 512):
                    ps = ppool.tile([128, 512], FP32, name="p", tag="p")
                    for cc in range(2):
                        nc.tensor.matmul(
                            ps[:],
                            lhsT=w_sb["wq"][cc][:, m * 128:(m + 1) * 128],
                            rhs=t2T[cc][:, nn * 512:(nn + 1) * 512],
                            start=(cc == 0), stop=(cc == 1))
                    evac(m * 2 + nn, qT[m][:, nn * 512:(nn + 1) * 512],
                         ps[:])
            emit_s_exp(0, 0, dve_busy=True)
            emit_s_exp(0, 1, dve_busy=True)
            emit_s_exp(0, 2, dve_busy=True)
            emit_s_exp(0, 3, dve_busy=True)
            for kc in range(16):
                ps = ppool.tile([128, C], FP32, name="p", tag="p")
                for cc in range(2):
                    nc.tensor.matmul(
                        ps[:],
                        lhsT=t1T[cc][:, kc * 128:(kc + 1) * 128],
                        rhs=w_sb["wv"][cc][:],
                        start=(cc == 0), stop=(cc == 1))
                v4 = v_sb[kc // 2][:].rearrange("p (ko h e) -> p ko h e",
                                                ko=2, h=4)
                evac(kc, v4[:, kc % 2, :, 1:65],
                     ps[:].rearrange("p (h e) -> p h e", e=64))

        def emit_xo(m, sc, xo_ps):
            # DoubleRow fp8 matmul: contracts 256 keys (2 ko sub-chunks)
            # per instruction, halving the xo stream count vs bf16 K=128.
            v4 = v_sb[sc][:].rearrange("p (ko h e) -> p ko h e", ko=2, h=4)
            sup = sup_map[(m, sc)]
            s3 = [t[:].rearrange("p (ko q) -> p ko q", ko=2) for t in sup]
            for j in range(Q // 512):
                for hh in range(2):
                    h = 2 * m + hh
                    nc.tensor.matmul(
                        xo_ps[hh][0:80, j * 512:(j + 1) * 512],
                        lhsT=v4[:, :, h, :],
                        rhs=s3[j][:, :, hh * 512:(hh + 1) * 512],
                        start=(sc == 0), stop=(sc == 7),
                        perf_mode=mybir.MatmulPerfMode.DoubleRow)

        xopool = attn_ctx.enter_context(
            tc.tile_pool(name="xopsum", bufs=1, space="PSUM"))

        osb = ctx.enter_context(tc.tile_pool(name="osb", bufs=2))
        o_ps = None

        for m in range(2):  # head pair (2m, 2m+1)
            nc.gpsimd.tensor_add(xT[m][:], t2T[m][:], qT[m][:])

            xo_ps = [xopool.tile([80, Q], FP32, name=f"xo{hh}", tag=f"xo{hh}")
                     for hh in range(2)]

            for sc in range(8):
                # stay one super-chunk of S/exp ahead of the DoubleRow
                # consumer so the exp engines chew tile sc+1 while the PE
                # streams xo of tile sc
                while emitted[m] < min(2 * sc + 4, 16):
                    emit_s_exp(m, emitted[m])
                emit_xo(m, sc, xo_ps)

            if m == 0:
                # keep ACT fed across the pair boundary: next pair's first
                # S/exp pairs go out before this pair's normalization chain
                emit_s_exp(1, 0, dve_busy=True)
                emit_s_exp(1, 1, dve_busy=True)
                emit_s_exp(1, 2, dve_busy=True)
            else:
                # final-projection pass A, TRANSPOSED: accumulate
                # oT[c, q] = Wp^T (t2+q)^T + pair-0 head contributions in
                # PSUM (tags sq0/sq1, free once the kc=15 exps have read
                # them).  Channels live on partitions, so every matmul
                # streams a fat N=512 and the bias can ride evacuation as
                # a per-partition scalar.  Runs on the PE while
                # DVE/GpSimd normalize pair 1 below; pass B accumulates
                # heads 2-3 into the same PSUM after normalization.
                o_ps = [spool.tile([128, Q], FP32, name=f"oT{half}",
                                   tag=f"sq{half}") for half in range(2)]
                for half in range(2):
                    hsl = slice(half * 128, (half + 1) * 128)
                    for j in range(Q // 512):
                        out_sl = o_ps[half][:, j * 512:(j + 1) * 512]
                        for cc in range(2):
                            nc.tensor.matmul(
                                out_sl,
                                lhsT=w_sb["wp"][cc][:, hsl],
                                rhs=xT[cc][:, j * 512:(j + 1) * 512],
                                start=(cc == 0), stop=False)
                        for h in range(2):
                            nc.tensor.matmul(
                                out_sl,
                                lhsT=wp_h[h][:, hsl],
                                rhs=xon[h][:, j * 512:(j + 1) * 512],
                                start=False, stop=False)

            # normalize: row 0 of xo_ps[hh] = sum_k exp(S)
            for hh in range(2):
                recip = npool.tile([1, Q], FP32, name=f"recip{hh}",
                                   tag=f"recip{hh}")
                nc.vector.reciprocal_approx_fast(recip[:, :],
                                                 xo_ps[hh][0:1, :])
                bc_sb = npool.tile([65, Q], FP32, name=f"bc{hh}",
                                   tag=f"bc{hh}")
                nc.gpsimd.partition_broadcast(bc_sb[:], recip[:])
                nc.vector.tensor_mul(xon[2 * m + hh][:], xo_ps[hh][0:65, :],
                                     bc_sb[:])

        # final-projection pass B: accumulate pair-1 heads into the
        # transposed PSUM accumulators, then evacuate with the bias added
        # per-partition (DVE and ACT alternate) and store oT.
        for half in range(2):
            hsl = slice(half * 128, (half + 1) * 128)
            for j in range(Q // 512):
                out_sl = o_ps[half][:, j * 512:(j + 1) * 512]
                for h in range(2, 4):
                    nc.tensor.matmul(
                        out_sl,
                        lhsT=wp_h[h][:, hsl],
                        rhs=xon[h][:, j * 512:(j + 1) * 512],
                        start=False, stop=(h == 3))
                o_sb = osb.tile([128, 512], FP32, name="o", tag=f"o{j}")
                if j == 0:
                    nc.vector.tensor_scalar_add(o_sb[:], out_sl, bpT[half][:])
                else:
                    nc.scalar.activation(o_sb[:], out_sl, AF.Identity,
                                         bias=bpT[half][:])
                dmae[(half + j) % 2].dma_start(
                    out=out_d[hsl, j * 512:(j + 1) * 512],
                    in_=o_sb[:])

        attn_ctx.close()

    nc.finalize()
    return nc


def _get_nc():
    if "nc" not in _CACHE:
        _CACHE["nc"] = build_nc()
    return _CACHE["nc"]


def _bf16(a):
    import ml_dtypes

    return np.ascontiguousarray(a.astype(ml_dtypes.bfloat16))


def make_in_maps(t2_grad, t1, Wq, Wkv, Wproj, bproj):
    t2 = np.asarray(t2_grad, dtype=np.float32)
    t1 = np.asarray(t1, dtype=np.float32)
    wq = _bf16(np.asarray(Wq, dtype=np.float32))
    wk = _bf16(np.ascontiguousarray(Wkv[:, :C], dtype=np.float32))
    wv = _bf16(np.ascontiguousarray(Wkv[:, C:], dtype=np.float32))
    wp = _bf16(np.asarray(Wproj, dtype=np.float32))
    bpT = np.ascontiguousarray(
        np.asarray(bproj, dtype=np.float32).reshape(C, 1))
    in_maps = []
    for c in range(NCORES):
        b, qh = c // 2, c % 2
        in_maps.append({
            "t1T": _bf16(t1[b].T),
            "t2T": _bf16(t2[b].T[:, qh * Q:(qh + 1) * Q]),
            "wq": wq, "wk": wk, "wv": wv, "wp": wp, "bpT": bpT,
        })
    return in_maps


def kernel(t2_grad, t1, Wq, Wkv, Wproj, bproj, gamma, _trace=False,
           _use_fp32r=True):
    gamma = np.asarray(gamma)
    if float(np.abs(gamma).max()) != 0.0:
        # LAM block is only the identity for gamma == 0; fall back to a
        # host reference for the general case (not exercised by the
        # reference setup_inputs, which fixes gamma = 0).
        return _host_reference(t2_grad, t1, Wq, Wkv, Wproj, bproj, gamma)

    nc = _get_nc()
    in_maps = make_in_maps(t2_grad, t1, Wq, Wkv, Wproj, bproj)
    res = run_bass_kernel_spmd(nc, in_maps, list(range(NCORES)), trace=_trace)
    out = np.empty((B, N, C), dtype=np.float32)
    for c in range(NCORES):
        b, qh = c // 2, c % 2
        out[b, qh * Q:(qh + 1) * Q, :] = res.results[c]["out"].T
    if _trace:
        _CACHE["last_result"] = res
    return out


def _host_reference(t2_grad, t1, Wq, Wkv, Wproj, bproj, gamma):
    t2 = np.asarray(t2_grad, dtype=np.float64)
    t1 = np.asarray(t1, dtype=np.float64)
    Wq = np.asarray(Wq, dtype=np.float64)
    Wkv = np.asarray(Wkv, dtype=np.float64)
    Wproj = np.asarray(Wproj, dtype=np.float64)
    bproj = np.asarray(bproj, dtype=np.float64)
    g = float(np.asarray(gamma).reshape(-1)[0])
    q = (t2 @ Wq).reshape(B, N, H, D).transpose(0, 2, 1, 3)
    kv = (t1 @ Wkv).reshape(B, N, 2, H, D).transpose(2, 0, 3, 1, 4)
    k, v = kv[0], kv[1]
    s = np.einsum('bhnd,bhmd->bhnm', q, k) * SCALE
    s = s - s.max(axis=-1, keepdims=True)
    p = np.exp(s)
    p /= p.sum(axis=-1, keepdims=True)
    x = np.einsum('bhnm,bhmd->bhnd', p, v)
    xp = x.transpose(0, 3, 1, 2).reshape(B, D, H * N)
    energy = xp @ xp.transpose(0, 2, 1)
    energy = energy - energy.max(axis=-1, keepdims=True)
    att = np.exp(energy)
    att /= att.sum(axis=-1, keepdims=True)
    lam_out = (att @ xp).reshape(B, D, H, N)
    lam_out = g * lam_out + xp.reshape(B, D, H, N)
    x = lam_out.transpose(0, 2, 3, 1)
    xo = x.transpose(0, 2, 1, 3).reshape(B, N, C) \
        + q.transpose(0, 2, 1, 3).reshape(B, N, C)
    return ((t2 + xo) @ Wproj + bproj).astype(np.float32)

